# revision 1
# baseline (speedup 1.0000x reference)
"""Trainium2 Bass kernel for nn_LoRAAQExpert (AQLM-style 2-codebook VQ MLP + LoRA).

Sharding: tensor-parallel over 8 cores — column-parallel gate/up (each core owns
INTER/8 = 1376 output features of both experts), row-parallel down (each core's
mid slice feeds its 1376-column slice of W_down), ReduceScatter of the f32
partial outputs over the token dim.  Matmuls run in bf16 with f32 PSUM
accumulation via the tile_matmul library kernel (DMA-transposed x/W tile loads);
silu*up fused on ACT+DVE; LoRA computed per-core (scaled by 1/8 so the
ReduceScatter sum restores it) with A/B pre-transposed host-side.  Weight
dequantization (codebook gather + scale fold, incl. the 0.01 output scale into
W_down) happens host-side during input sharding: the device indirect-DMA path
only supports one offset per partition per instruction (verified on HW), which
cannot sustain the 4.2M random 32B gathers/core this problem needs.
"""

import sys

sys.path.insert(0, "/opt/trn_rl_repo")

from contextlib import ExitStack

import numpy as np
import ml_dtypes

from concourse import bacc, bass, mybir, tile
from concourse import bass_utils
from concourse.bass import IndirectOffsetOnAxis
from concourse.kernels.tile_matmul import matmul_tile_kernel

F32 = mybir.dt.float32
BF16 = mybir.dt.bfloat16
I32 = mybir.dt.int32

P = 128
RS_CHUNKS = 4


def full_cfg():
    return dict(
        HID=4096, INTER=11008, GS=8, KCB=65536, TOK=8192, R=128, NC=8,
        OPAD=1536,  # per-core gate/up output shard (1376) padded to a 512 multiple
    )


def derived(cfg):
    d = dict(cfg)
    d["G"] = cfg["HID"] // cfg["GS"]          # input groups for gate/up
    d["OSH"] = cfg["INTER"] // cfg["NC"]      # real per-core o-shard
    d["GDR"] = d["OSH"] // cfg["GS"]          # real down groups per core
    d["GDPAD"] = cfg["OPAD"] // cfg["GS"]     # padded down groups
    d["TSH"] = cfg["TOK"] // cfg["NC"]        # output token shard
    return d


def _dequant_expert(ctx, tc, pools, idx0_t, idx1_t, cb0_t, cb1_t, scale_sb,
                    w_dst, n_rows, n_real_rows, n_groups, n_real_groups):
    """Dequantize one expert's weight shard into DRAM bf16.

    idx*_t : DRAM int32 [n_rows_idx, n_groups_idx] (only real region is read)
    cb*_t  : DRAM f32 [KCB, GS]
    scale_sb : SBUF f32 [1, n_groups*GS] input-feature scales (already includes
               any constant folding), broadcast over partitions.
    w_dst  : DRAM bf16 [n_rows, n_groups*GS]
    """
    nc = tc.nc
    gs = cb0_t.shape[-1]
    ncols = n_groups * gs
    nrealc = n_real_groups * gs
    idx_pool, w_pool, ws_pool = pools

    # zero-fill the padded W rows once (DRAM destination: no partition limits)
    if n_real_rows < n_rows:
        zt = ws_pool.tile([P, ncols], BF16, tag="ws")
        nc.vector.memset(zt[:], 0.0)
        r = n_real_rows
        while r < n_rows:
            n = min(P, n_rows - r)
            nc.sync.dma_start(w_dst[r:r + n, :], zt[0:n, :])
            r += n

    for s in range((n_real_rows + P - 1) // P):
        r0 = s * P
        nreal = min(n_real_rows - r0, P)
        it0 = idx_pool.tile([P, n_real_groups], I32, tag="idx0")
        it1 = idx_pool.tile([P, n_real_groups], I32, tag="idx1")
        if nreal < P:
            # unread pad rows gather entry 0 (their outputs are never shipped)
            nc.vector.memset(it0[:], 0)
            nc.vector.memset(it1[:], 0)
        nc.sync.dma_start(it0[0:nreal, :], idx0_t[r0:r0 + nreal, :])
        nc.sync.dma_start(it1[0:nreal, :], idx1_t[r0:r0 + nreal, :])
        wt0 = w_pool.tile([P, n_real_groups, gs], F32, tag="wt0")
        wt1 = w_pool.tile([P, n_real_groups, gs], F32, tag="wt1")
        nc.gpsimd.indirect_dma_start(
            out=wt0[:],
            out_offset=None,
            in_=cb0_t[:],
            in_offset=IndirectOffsetOnAxis(ap=it0[:], axis=0),
        )
        nc.gpsimd.indirect_dma_start(
            out=wt1[:],
            out_offset=None,
            in_=cb1_t[:],
            in_offset=IndirectOffsetOnAxis(ap=it1[:], axis=0),
        )
        wsum = w_pool.tile([P, nrealc], F32, tag="wsum")
        nc.vector.tensor_tensor(
            out=wsum[:],
            in0=wt0[:].rearrange("p g e -> p (g e)"),
            in1=wt1[:].rearrange("p g e -> p (g e)"),
            op=mybir.AluOpType.add,
        )
        ws = ws_pool.tile([P, ncols], BF16, tag="ws")
        if nrealc < ncols:
            nc.vector.memset(ws[:, nrealc:], 0.0)
        nc.vector.tensor_tensor(
            out=ws[:, 0:nrealc],
            in0=wsum[:],
            in1=scale_sb[:, 0:nrealc],
            op=mybir.AluOpType.mult,
        )
        nc.sync.dma_start(w_dst[r0:r0 + nreal, :], ws[0:nreal, :])


def build(cfg, use_collective=True, debug_outs=False):
    d = derived(cfg)
    HID, GS, KCB, TOK, R, NC, OPAD = (cfg[k] for k in
                                      ("HID", "GS", "KCB", "TOK", "R", "NC", "OPAD"))
    G, OSH, GDR, GDPAD, TSH = (d[k] for k in ("G", "OSH", "GDR", "GDPAD", "TSH"))

    nc = bacc.Bacc("TRN2", target_bir_lowering=False, debug=False,
                   enable_asserts=False, num_devices=NC)

    xb = nc.dram_tensor("xb", [TOK, HID], BF16, kind="ExternalInput")
    wgu_in = nc.dram_tensor("wgu_in", [2 * OPAD, HID], BF16, kind="ExternalInput")
    wd_in = nc.dram_tensor("wd_in", [HID, OPAD], BF16, kind="ExternalInput")
    at = nc.dram_tensor("at", [HID, R], BF16, kind="ExternalInput")
    bt = nc.dram_tensor("bt", [R, HID], BF16, kind="ExternalInput")
    out_rows = TSH if use_collective else TOK
    out = nc.dram_tensor("out", [out_rows, HID], F32, kind="ExternalOutput")
    if debug_outs:
        dbg_wgu = nc.dram_tensor("dbg_wgu", [2 * OPAD, HID], BF16, kind="ExternalOutput")
        dbg_gu = nc.dram_tensor("dbg_gu", [TOK, 2 * OPAD], F32, kind="ExternalOutput")
        dbg_mid = nc.dram_tensor("dbg_mid", [TOK, OPAD], BF16, kind="ExternalOutput")
        dbg_lacc = nc.dram_tensor("dbg_lacc", [TOK, HID], F32, kind="ExternalOutput")

    with tile.TileContext(nc) as tc:
        with ExitStack() as ctx:
            dram = ctx.enter_context(tc.tile_pool(name="dram", bufs=1, space="DRAM"))
            gu = dram.tile([TOK, 2 * OPAD], BF16)
            mid = dram.tile([TOK, OPAD], BF16)
            acc = dram.tile([TOK, HID], F32)
            lacc = dram.tile([TOK, HID], F32)
            lmidT = dram.tile([R, TOK], BF16)
            rs = dram.tile([TSH, HID], F32)

            # ---- lora: lmidT = A^T(stat) x^T(mov);  acc = lmidT^T @ B^T ----
            matmul_tile_kernel(tc,
                               kxm_ap=at.ap(),
                               kxn_ap=xb.ap(),
                               mxn_ap=lmidT[:],
                               transpose_kxn=True)
            matmul_tile_kernel(tc,
                               kxm_ap=lmidT[:],
                               kxn_ap=bt.ap(),
                               mxn_ap=lacc[:])

            # ---- gate/up matmul: gu[t, 2*OPAD] = x @ Wgu^T ----
            matmul_tile_kernel(tc,
                               kxm_ap=xb.ap(),
                               kxn_ap=wgu_in.ap(),
                               mxn_ap=gu[:],
                               transpose_kxm=True,
                               transpose_kxn=True)

            # ---- mid = silu(gate) * up  (bf16) ----
            with tc.tile_pool(name="si_in", bufs=3) as si_in, \
                 tc.tile_pool(name="si_t", bufs=3) as si_t, \
                 tc.tile_pool(name="si_o", bufs=3) as si_o:
                for s in range(TOK // P):
                    t0 = s * P
                    gt = si_in.tile([P, 2 * OPAD], BF16, tag="gt")
                    nc.sync.dma_start(gt[:], gu[t0:t0 + P, :])
                    sl = si_t.tile([P, OPAD], BF16, tag="sl")
                    nc.scalar.activation(sl[:], gt[:, 0:OPAD],
                                         mybir.ActivationFunctionType.Silu)
                    md = si_o.tile([P, OPAD], BF16, tag="md")
                    nc.vector.tensor_tensor(out=md[:], in0=sl[:],
                                            in1=gt[:, OPAD:2 * OPAD],
                                            op=mybir.AluOpType.mult)
                    nc.sync.dma_start(mid[t0:t0 + P, :], md[:])

            # ---- down matmul accumulated onto lora partial ----
            matmul_tile_kernel(tc,
                               kxm_ap=mid[:],
                               kxn_ap=wd_in.ap(),
                               mxn_ap=acc[:],
                               transpose_kxm=True,
                               transpose_kxn=True,
                               accumulate_ap=lacc[:])

            # ---- ReduceScatter over the 8 cores, then emit our token shard ----
            if use_collective:
                ch = TOK // RS_CHUNKS
                och = ch // NC
                for k in range(RS_CHUNKS):
                    nc.gpsimd.collective_compute(
                        "ReduceScatter",
                        mybir.AluOpType.add,
                        replica_groups=[list(range(NC))],
                        ins=[acc[k * ch:(k + 1) * ch, :].opt()],
                        outs=[rs[k * och:(k + 1) * och, :].opt()],
                    )
                nc.sync.dma_start(out.ap(), rs[:])
            else:
                nc.sync.dma_start(out.ap(), acc[:])
            if debug_outs:
                nc.sync.dma_start(dbg_wgu.ap(), wgu_in.ap())
                nc.sync.dma_start(dbg_gu.ap(), gu[:])
                nc.sync.dma_start(dbg_mid.ap(), mid[:])
                nc.sync.dma_start(dbg_lacc.ap(), lacc[:])

    nc.compile()
    return nc


def shard_inputs(cfg, inputs):
    """Build per-core in_maps from the full-size input dict (host dequant)."""
    d = derived(cfg)
    HID, GS, KCB, TOK, R, NC, OPAD = (cfg[k] for k in
                                      ("HID", "GS", "KCB", "TOK", "R", "NC", "OPAD"))
    G, OSH, GDR, GDPAD = (d[k] for k in ("G", "OSH", "GDR", "GDPAD"))
    bf16 = ml_dtypes.bfloat16

    x = np.asarray(inputs["x"], np.float32).reshape(TOK, HID)
    xb = np.ascontiguousarray(x.astype(bf16))

    gcb = np.asarray(inputs["gate_codebooks"], np.float32)
    ucb = np.asarray(inputs["up_codebooks"], np.float32)
    dcb = np.asarray(inputs["down_codebooks"], np.float32)
    gi = np.asarray(inputs["gate_indices"], np.int32)
    ui = np.asarray(inputs["up_indices"], np.int32)
    di = np.asarray(inputs["down_indices"], np.int32)
    gs_ = np.asarray(inputs["gate_scales"], np.float32)
    us_ = np.asarray(inputs["up_scales"], np.float32)
    ds_ = np.asarray(inputs["down_scales"], np.float32)
    at = np.ascontiguousarray(np.asarray(inputs["lora_A"], np.float32).T.astype(bf16))
    SCALING = 256.0 / 128.0
    bt = np.ascontiguousarray(
        (np.asarray(inputs["lora_B"], np.float32).T * (SCALING / NC)).astype(bf16))

    def dq(idx, cb, scale):
        # idx [O, Gn, 2] -> [O, Gn*GS] f32 times per-input-feature scale
        w = cb[0][idx[:, :, 0]] + cb[1][idx[:, :, 1]]
        return w.reshape(idx.shape[0], -1) * scale

    in_maps = []
    for c in range(NC):
        wg = dq(gi[c * OSH:(c + 1) * OSH], gcb, gs_)
        wu = dq(ui[c * OSH:(c + 1) * OSH], ucb, us_)
        wgu = np.zeros((2 * OPAD, HID), bf16)
        wgu[:OSH] = wg.astype(bf16)
        wgu[OPAD:OPAD + OSH] = wu.astype(bf16)
        # down: rows = HID outputs, cols = this core's 1376 inter features;
        # fold down_scales (per inter feature) and the 0.01 output scale in.
        wdd = dq(di[:, c * GDR:(c + 1) * GDR, :], dcb,
                 ds_[c * OSH:(c + 1) * OSH] * 0.01)
        wd = np.zeros((HID, OPAD), bf16)
        wd[:, :OSH] = wdd.astype(bf16)
        in_maps.append({
            "xb": xb,
            "wgu_in": np.ascontiguousarray(wgu),
            "wd_in": np.ascontiguousarray(wd),
            "at": at,
            "bt": bt,
        })
    return in_maps


_NC_CACHE = {}


def _compiled(cfg):
    key = tuple(sorted(cfg.items()))
    if key not in _NC_CACHE:
        _NC_CACHE[key] = build(cfg)
    return _NC_CACHE[key]


def run(cfg, inputs, trace=False):
    nc = _compiled(cfg)
    in_maps = shard_inputs(cfg, inputs)
    res = bass_utils.run_bass_kernel_spmd(
        nc, in_maps, core_ids=list(range(cfg["NC"])), trace=trace)
    return assemble(cfg, res), res


def assemble(cfg, res):
    """Reorder the chunked-ReduceScatter per-core shards into token order."""
    TOK, NC, HID = cfg["TOK"], cfg["NC"], cfg["HID"]
    ch = TOK // RS_CHUNKS
    och = ch // NC
    outs = np.empty((TOK, HID), np.float32)
    for c in range(NC):
        p = res.results[c]["out"]
        for k in range(RS_CHUNKS):
            outs[k * ch + c * och:k * ch + (c + 1) * och] = p[k * och:(k + 1) * och]
    return outs


def kernel(**inputs):
    cfg = full_cfg()
    x = np.asarray(inputs["x"])
    outs, _ = run(cfg, inputs)
    return outs.reshape(x.shape[0], x.shape[1], cfg["HID"]).astype(np.float32)



# revision 2
# speedup vs baseline: 1.7824x; 1.7824x over previous
"""Trainium2 Bass kernel for nn_LoRAAQExpert (AQLM-style 2-codebook VQ MLP + LoRA).

Sharding: tensor-parallel over 8 cores — column-parallel gate/up (each core owns
INTER/8 = 1376 output features of both experts), row-parallel down, ReduceScatter
of the f32 partial outputs over the token dim.  x is shipped token-sharded
(TOK/8 rows per core) and AllGathered on device, weights are dequantized
host-side (device indirect-DMA gather can't sustain 4.2M random 16B gathers)
and shipped unpadded bf16; padding to the 512-multiple matmul shape happens
on device.  LoRA is computed per-core (scaled by 1/8 so the ReduceScatter sum
restores it).  Output is emitted bf16 to halve the device->host fetch.

The host<->device link (axon tunnel, ~40-70 MB/s) dominates end-to-end time,
so the kernel is organized to minimize bytes shipped per invocation.
"""

import os
import sys

sys.path.insert(0, "/opt/trn_rl_repo")

from contextlib import ExitStack

import numpy as np
import ml_dtypes

from concourse import bacc, bass, mybir, tile
from concourse.kernels.tile_matmul import matmul_tile_kernel

F32 = mybir.dt.float32
BF16 = mybir.dt.bfloat16
I32 = mybir.dt.int32

P = 128
RS_CHUNKS = 4

# "nki" (target_bir_lowering=True, no zero output buffers shipped) or
# "exec" (baseline lowering; zero output buffers shipped and donated)
MODE = os.environ.get("BASSV2_MODE", "nki")


def full_cfg():
    return dict(
        HID=4096, INTER=11008, GS=8, KCB=65536, TOK=8192, R=128, NC=8,
        OPAD=1536,  # per-core gate/up output shard (1376) padded to a 512 multiple
    )


def derived(cfg):
    d = dict(cfg)
    d["G"] = cfg["HID"] // cfg["GS"]          # input groups for gate/up
    d["OSH"] = cfg["INTER"] // cfg["NC"]      # real per-core o-shard (1376)
    d["GDR"] = d["OSH"] // cfg["GS"]          # real down groups per core
    d["TSH"] = cfg["TOK"] // cfg["NC"]        # token shard
    return d


def build(cfg):
    d = derived(cfg)
    HID, TOK, R, NC, OPAD = (cfg[k] for k in ("HID", "TOK", "R", "NC", "OPAD"))
    OSH, TSH = d["OSH"], d["TSH"]

    nc = bacc.Bacc("TRN2", target_bir_lowering=(MODE == "nki"), debug=False,
                   enable_asserts=False, num_devices=NC)

    xs = nc.dram_tensor("xs", [TSH, HID], BF16, kind="ExternalInput")
    wgu_in = nc.dram_tensor("wgu_in", [2 * OSH, HID], BF16, kind="ExternalInput")
    wd_in = nc.dram_tensor("wd_in", [HID, OSH], BF16, kind="ExternalInput")
    at = nc.dram_tensor("at", [HID, R], BF16, kind="ExternalInput")
    bt = nc.dram_tensor("bt", [R, HID], BF16, kind="ExternalInput")
    out = nc.dram_tensor("out", [TSH, HID], BF16, kind="ExternalOutput")

    with tile.TileContext(nc) as tc:
        with ExitStack() as ctx:
            dram = ctx.enter_context(tc.tile_pool(name="dram", bufs=1, space="DRAM"))
            xstage = dram.tile([TSH, HID], BF16)
            xfull = dram.tile([TOK, HID], BF16)
            wgu = dram.tile([2 * OPAD, HID], BF16)
            wd = dram.tile([HID, OPAD], BF16)
            gu = dram.tile([TOK, 2 * OPAD], BF16)
            mid = dram.tile([TOK, OPAD], BF16)
            acc = dram.tile([TOK, HID], F32)
            lacc = dram.tile([TOK, HID], F32)
            lmidT = dram.tile([R, TOK], BF16)
            rs = dram.tile([TSH, HID], F32)

            # ---- AllGather the token shards into the full x ----
            # (collectives cannot read IO tensors; stage the shard first)
            nc.sync.dma_start(xstage[:], xs.ap())
            nc.gpsimd.collective_compute(
                "AllGather",
                mybir.AluOpType.bypass,
                replica_groups=[list(range(NC))],
                ins=[xstage[:].opt()],
                outs=[xfull[:].opt()],
            )

            # ---- pad the shipped weights to the 512-multiple matmul shapes ----
            with tc.tile_pool(name="zpool", bufs=1) as zpool:
                zt = zpool.tile([P, HID], BF16, tag="z")
                nc.vector.memset(zt[:], 0.0)
                npad = OPAD - OSH  # 160
                # wgu pad rows: [OSH:OPAD] and [OPAD+OSH:2*OPAD]
                for base in (0, OPAD):
                    r = base + OSH
                    while r < base + OPAD:
                        n = min(P, base + OPAD - r)
                        nc.sync.dma_start(wgu[r:r + n, :], zt[0:n, :])
                        r += n
                # wd pad cols: [:, OSH:OPAD]
                for r in range(0, HID, P):
                    nc.sync.dma_start(wd[r:r + P, OSH:OPAD], zt[0:P, 0:npad])
            # real weight regions (DRAM -> DRAM copies)
            nc.sync.dma_start(wgu[0:OSH, :], wgu_in[0:OSH, :])
            nc.sync.dma_start(wgu[OPAD:OPAD + OSH, :], wgu_in[OSH:2 * OSH, :])
            nc.sync.dma_start(wd[:, 0:OSH], wd_in[:, :])

            # ---- lora: lmidT = A^T(stat) x^T(mov);  lacc = lmidT^T @ B^T ----
            matmul_tile_kernel(tc,
                               kxm_ap=at.ap(),
                               kxn_ap=xfull[:],
                               mxn_ap=lmidT[:],
                               transpose_kxn=True)
            matmul_tile_kernel(tc,
                               kxm_ap=lmidT[:],
                               kxn_ap=bt.ap(),
                               mxn_ap=lacc[:])

            # ---- gate/up matmul: gu[t, 2*OPAD] = x @ Wgu^T ----
            matmul_tile_kernel(tc,
                               kxm_ap=xfull[:],
                               kxn_ap=wgu[:],
                               mxn_ap=gu[:],
                               transpose_kxm=True,
                               transpose_kxn=True)

            # ---- mid = silu(gate) * up  (bf16) ----
            with tc.tile_pool(name="si_in", bufs=3) as si_in, \
                 tc.tile_pool(name="si_t", bufs=3) as si_t, \
                 tc.tile_pool(name="si_o", bufs=3) as si_o:
                for s in range(TOK // P):
                    t0 = s * P
                    gt = si_in.tile([P, 2 * OPAD], BF16, tag="gt")
                    nc.sync.dma_start(gt[:], gu[t0:t0 + P, :])
                    sl = si_t.tile([P, OPAD], BF16, tag="sl")
                    nc.scalar.activation(sl[:], gt[:, 0:OPAD],
                                         mybir.ActivationFunctionType.Silu)
                    md = si_o.tile([P, OPAD], BF16, tag="md")
                    nc.vector.tensor_tensor(out=md[:], in0=sl[:],
                                            in1=gt[:, OPAD:2 * OPAD],
                                            op=mybir.AluOpType.mult)
                    nc.sync.dma_start(mid[t0:t0 + P, :], md[:])

            # ---- down matmul accumulated onto lora partial ----
            matmul_tile_kernel(tc,
                               kxm_ap=mid[:],
                               kxn_ap=wd[:],
                               mxn_ap=acc[:],
                               transpose_kxm=True,
                               transpose_kxn=True,
                               accumulate_ap=lacc[:])

            # ---- ReduceScatter over the 8 cores ----
            ch = TOK // RS_CHUNKS
            och = ch // NC
            for k in range(RS_CHUNKS):
                nc.gpsimd.collective_compute(
                    "ReduceScatter",
                    mybir.AluOpType.add,
                    replica_groups=[list(range(NC))],
                    ins=[acc[k * ch:(k + 1) * ch, :].opt()],
                    outs=[rs[k * och:(k + 1) * och, :].opt()],
                )

            # ---- emit the token shard as bf16 ----
            with tc.tile_pool(name="cv_in", bufs=3) as cv_in, \
                 tc.tile_pool(name="cv_o", bufs=3) as cv_o:
                for s in range(TSH // P):
                    t0 = s * P
                    rt = cv_in.tile([P, HID], F32, tag="rt")
                    nc.sync.dma_start(rt[:], rs[t0:t0 + P, :])
                    ot = cv_o.tile([P, HID], BF16, tag="ot")
                    nc.scalar.activation(ot[:], rt[:],
                                         mybir.ActivationFunctionType.Copy)
                    nc.sync.dma_start(out.ap()[t0:t0 + P, :], ot[:])

    nc.compile()
    return nc


def shard_inputs(cfg, inputs):
    """Build per-core in_maps from the full-size input dict (host dequant)."""
    d = derived(cfg)
    HID, TOK, R, NC = (cfg[k] for k in ("HID", "TOK", "R", "NC"))
    OSH, GDR, TSH = d["OSH"], d["GDR"], d["TSH"]
    bf16 = ml_dtypes.bfloat16

    x = np.asarray(inputs["x"], np.float32).reshape(TOK, HID)
    xb = np.ascontiguousarray(x.astype(bf16))

    gcb = np.asarray(inputs["gate_codebooks"], np.float32)
    ucb = np.asarray(inputs["up_codebooks"], np.float32)
    dcb = np.asarray(inputs["down_codebooks"], np.float32)
    gi = np.asarray(inputs["gate_indices"], np.int32)
    ui = np.asarray(inputs["up_indices"], np.int32)
    di = np.asarray(inputs["down_indices"], np.int32)
    gs_ = np.asarray(inputs["gate_scales"], np.float32)
    us_ = np.asarray(inputs["up_scales"], np.float32)
    ds_ = np.asarray(inputs["down_scales"], np.float32)
    at = np.ascontiguousarray(np.asarray(inputs["lora_A"], np.float32).T.astype(bf16))
    SCALING = 256.0 / 128.0
    bt = np.ascontiguousarray(
        (np.asarray(inputs["lora_B"], np.float32).T * (SCALING / NC)).astype(bf16))

    def dq(idx, cb, scale):
        # idx [O, Gn, 2] -> [O, Gn*GS] f32 times per-input-feature scale
        w = cb[0][idx[:, :, 0]] + cb[1][idx[:, :, 1]]
        return w.reshape(idx.shape[0], -1) * scale

    in_maps = []
    for c in range(NC):
        wg = dq(gi[c * OSH:(c + 1) * OSH], gcb, gs_)
        wu = dq(ui[c * OSH:(c + 1) * OSH], ucb, us_)
        wgu = np.empty((2 * OSH, HID), bf16)
        wgu[:OSH] = wg.astype(bf16)
        wgu[OSH:] = wu.astype(bf16)
        # down: rows = HID outputs, cols = this core's 1376 inter features;
        # fold down_scales (per inter feature) and the 0.01 output scale in.
        wdd = dq(di[:, c * GDR:(c + 1) * GDR, :], dcb,
                 ds_[c * OSH:(c + 1) * OSH] * 0.01)
        in_maps.append({
            "xs": np.ascontiguousarray(xb[c * TSH:(c + 1) * TSH]),
            "wgu_in": np.ascontiguousarray(wgu),
            "wd_in": np.ascontiguousarray(wdd.astype(bf16)),
            "at": at,
            "bt": bt,
        })
    return in_maps


_NC_CACHE = {}
_RUNNER_CACHE = {}


def _compiled(cfg):
    key = tuple(sorted(cfg.items()))
    if key not in _NC_CACHE:
        _NC_CACHE[key] = build(cfg)
    return _NC_CACHE[key]


class Runner:
    """Dispatch a compiled Bass SPMD kernel across the 8 cores via PJRT,
    shipping each core's shard directly (no host-side concat) and, in nki
    mode, no zero-filled output donation buffers."""

    def __init__(self, nc, n_cores):
        import jax
        from jax.sharding import Mesh, PartitionSpec, NamedSharding
        from jax.experimental.shard_map import shard_map
        from concourse.bass2jax import (_bass_exec_p, install_neuronx_cc_hook,
                                        partition_id_tensor)

        install_neuronx_cc_hook()
        self.jax = jax
        self.nc = nc
        self.n_cores = n_cores
        partition_name = (nc.partition_id_tensor.name
                          if nc.partition_id_tensor else None)
        in_names, out_names, out_avals, zero_outs = [], [], [], []
        for alloc in nc.m.functions[0].allocations:
            if not isinstance(alloc, mybir.MemoryLocationSet):
                continue
            name = alloc.memorylocations[0].name
            if alloc.kind == "ExternalInput":
                if name != partition_name:
                    in_names.append(name)
            elif alloc.kind == "ExternalOutput":
                out_names.append(name)
                shape = tuple(alloc.tensor_shape)
                dtype = mybir.dt.np(alloc.dtype)
                out_avals.append(jax.core.ShapedArray(shape, dtype))
                zero_outs.append(np.zeros(shape, dtype))
        self.in_names = in_names
        self.out_names = out_names
        self.out_avals = out_avals
        self.zero_outs = zero_outs if MODE == "exec" else []
        n_params = len(in_names)
        n_outs = len(out_names)

        bind_in_names = list(in_names)
        if MODE == "exec":
            bind_in_names.extend(out_names)
        if partition_name is not None:
            bind_in_names.append(partition_name)

        def _body(*args):
            operands = list(args)
            if partition_name is not None:
                operands.append(partition_id_tensor())
            outs = _bass_exec_p.bind(
                *operands,
                out_avals=tuple(out_avals),
                in_names=tuple(bind_in_names),
                out_names=tuple(out_names),
                lowering_input_output_aliases=(),
                sim_require_finite=True,
                sim_require_nnan=True,
                nc=nc,
            )
            return tuple(outs)

        self.devices = jax.devices()[:n_cores]
        self.mesh = Mesh(np.asarray(self.devices), ("core",))
        self.pspec = PartitionSpec("core")
        self.sharding = NamedSharding(self.mesh, self.pspec)
        n_all = n_params + (n_outs if MODE == "exec" else 0)
        in_specs = (self.pspec,) * n_all
        out_specs = (self.pspec,) * n_outs
        donate = (tuple(range(n_params, n_params + n_outs))
                  if MODE == "exec" else ())
        self.fn = jax.jit(
            shard_map(_body, mesh=self.mesh, in_specs=in_specs,
                      out_specs=out_specs, check_rep=False),
            donate_argnums=donate, keep_unused=True)

    def _to_global(self, shards):
        jax = self.jax
        s = shards[0].shape
        global_shape = (self.n_cores * s[0],) + tuple(s[1:])
        parts = [jax.device_put(shards[c], self.devices[c])
                 for c in range(self.n_cores)]
        return jax.make_array_from_single_device_arrays(
            global_shape, self.sharding, parts)

    def __call__(self, in_maps):
        jax = self.jax
        args = [self._to_global([m[name] for m in in_maps])
                for name in self.in_names]
        for z in self.zero_outs:
            args.append(self._to_global(
                [z] + [np.zeros_like(z) for _ in range(self.n_cores - 1)]))
        out_arrs = self.fn(*args)
        out_arrs = jax.block_until_ready(out_arrs)
        res = []
        for c in range(self.n_cores):
            res.append({})
        for i, name in enumerate(self.out_names):
            g = np.asarray(out_arrs[i])
            s0 = self.out_avals[i].shape[0]
            for c in range(self.n_cores):
                res[c][name] = g[c * s0:(c + 1) * s0]
        return res


def _runner(cfg):
    key = tuple(sorted(cfg.items()))
    if key not in _RUNNER_CACHE:
        _RUNNER_CACHE[key] = Runner(_compiled(cfg), cfg["NC"])
    return _RUNNER_CACHE[key]


def assemble(cfg, results):
    """Reorder the chunked-ReduceScatter per-core shards into token order."""
    TOK, NC, HID = cfg["TOK"], cfg["NC"], cfg["HID"]
    ch = TOK // RS_CHUNKS
    och = ch // NC
    outs = np.empty((TOK, HID), np.float32)
    for c in range(NC):
        p = results[c]["out"].astype(np.float32)
        for k in range(RS_CHUNKS):
            outs[k * ch + c * och:k * ch + (c + 1) * och] = p[k * och:(k + 1) * och]
    return outs


def run(cfg, inputs):
    runner = _runner(cfg)
    in_maps = shard_inputs(cfg, inputs)
    res = runner(in_maps)
    return assemble(cfg, res), res


def kernel(**inputs):
    cfg = full_cfg()
    x = np.asarray(inputs["x"])
    outs, _ = run(cfg, inputs)
    return outs.reshape(x.shape[0], x.shape[1], cfg["HID"]).astype(np.float32)


# revision 3
# speedup vs baseline: 2.5819x; 1.4486x over previous
"""Trainium2 Bass kernel for nn_LoRAAQExpert (AQLM-style 2-codebook VQ MLP + LoRA).

v3: on-device weight dequantization.  Ships uint16 codebook indices + sharded
bf16 codebooks + scales instead of dequantized bf16 weights (74 MB total vs
271 MB), AllGathers the codebooks and token-sharded x on device, then
dequantizes via per-partition indirect-DMA gathers (one offset per partition
per instruction — the HW-verified limit) into the padded matmul weight
layouts.  Tensor-parallel gate/up + row-parallel down with chunked
ReduceScatter; LoRA computed per-core scaled by 1/8.  Output emitted bf16.

The host<->device axon tunnel (~40-70 MB/s) dominates end-to-end time, so
everything is organized to minimize bytes shipped per invocation.
"""

import os
import sys

sys.path.insert(0, "/opt/trn_rl_repo")

from contextlib import ExitStack

import numpy as np
import ml_dtypes

from concourse import bacc, bass, mybir, tile
from concourse.bass import IndirectOffsetOnAxis
from concourse.kernels.tile_matmul import matmul_tile_kernel

F32 = mybir.dt.float32
BF16 = mybir.dt.bfloat16
I32 = mybir.dt.int32
U16 = mybir.dt.uint16

P = 128
RS_CHUNKS = 4
MODE = os.environ.get("BASSV2_MODE", "nki")


def full_cfg():
    return dict(
        HID=4096, INTER=11008, GS=8, KCB=65536, TOK=8192, R=128, NC=8,
        OPAD=1536,  # per-core gate/up output shard (1376) padded to a 512 multiple
    )


def derived(cfg):
    d = dict(cfg)
    d["G"] = cfg["HID"] // cfg["GS"]          # gate/up groups per row (512)
    d["OSH"] = cfg["INTER"] // cfg["NC"]      # real per-core o-shard (1376)
    d["GDR"] = d["OSH"] // cfg["GS"]          # down groups per core (172)
    d["TSH"] = cfg["TOK"] // cfg["NC"]        # token shard (1024)
    d["CBS"] = cfg["KCB"] // cfg["NC"]        # codebook shard rows (8192)
    return d


def _dequant(nc, tc, ctx, idx_t, cb0, cb1, scale_sb, w_dst, w_col0,
             n_real_rows, G, GS, tag):
    """Gather-dequantize one expert: idx_t [rows, 2G] u16 -> w_dst bf16.

    w_dst is a DRAM AP; rows [0:n_real_rows], cols [w_col0 : w_col0+G*GS].
    One indirect-DMA gather per (row-tile, group, book): HW honors exactly
    one offset per partition per instruction.
    """
    GV = G * GS
    with tc.tile_pool(name=f"dqi{tag}", bufs=2) as ip, \
         tc.tile_pool(name=f"dqc{tag}", bufs=2) as icp, \
         tc.tile_pool(name=f"dqw{tag}", bufs=2) as wp, \
         tc.tile_pool(name=f"dqs{tag}", bufs=2) as sp, \
         tc.tile_pool(name=f"dqo{tag}", bufs=2) as op:
        for t in range((n_real_rows + P - 1) // P):
            r0 = t * P
            nr = min(P, n_real_rows - r0)
            i16 = ip.tile([P, 2 * G], U16, tag="i16")
            if nr < P:
                nc.vector.memset(i16[nr:P, :], 0)
            nc.sync.dma_start(i16[0:nr, :], idx_t[r0:r0 + nr, :])
            i32 = icp.tile([P, 2 * G], I32, tag="i32")
            nc.vector.tensor_copy(out=i32[:], in_=i16[:])
            wt = wp.tile([P, 2 * GV], BF16, tag="wt")
            for g in range(G):
                nc.gpsimd.indirect_dma_start(
                    out=wt[:, g * GS:(g + 1) * GS], out_offset=None,
                    in_=cb0,
                    in_offset=IndirectOffsetOnAxis(ap=i32[:, g:g + 1], axis=0))
            for g in range(G):
                nc.gpsimd.indirect_dma_start(
                    out=wt[:, GV + g * GS:GV + (g + 1) * GS], out_offset=None,
                    in_=cb1,
                    in_offset=IndirectOffsetOnAxis(
                        ap=i32[:, G + g:G + g + 1], axis=0))
            wsum = sp.tile([P, GV], F32, tag="wsum")
            nc.vector.tensor_tensor(out=wsum[:], in0=wt[:, 0:GV],
                                    in1=wt[:, GV:], op=mybir.AluOpType.add)
            ws = op.tile([P, GV], BF16, tag="ws")
            nc.vector.tensor_tensor(out=ws[:], in0=wsum[:], in1=scale_sb[:],
                                    op=mybir.AluOpType.mult)
            nc.sync.dma_start(
                w_dst[r0:r0 + nr, w_col0:w_col0 + GV], ws[0:nr, :])


def build(cfg):
    d = derived(cfg)
    HID, GS, KCB, TOK, R, NC, OPAD = (cfg[k] for k in
                                      ("HID", "GS", "KCB", "TOK", "R", "NC", "OPAD"))
    G, OSH, GDR, TSH, CBS = (d[k] for k in ("G", "OSH", "GDR", "TSH", "CBS"))

    nc = bacc.Bacc("TRN2", target_bir_lowering=(MODE == "nki"), debug=False,
                   enable_asserts=False, num_devices=NC)

    xs = nc.dram_tensor("xs", [TSH, HID], BF16, kind="ExternalInput")
    gi = nc.dram_tensor("gi", [OSH, 2 * G], U16, kind="ExternalInput")
    ui = nc.dram_tensor("ui", [OSH, 2 * G], U16, kind="ExternalInput")
    di = nc.dram_tensor("di", [HID, 2 * GDR], U16, kind="ExternalInput")
    cbsh = [nc.dram_tensor(f"cbsh{b}", [CBS, GS], BF16, kind="ExternalInput")
            for b in range(6)]  # g0 g1 u0 u1 d0 d1, row-sharded over cores
    gsc = nc.dram_tensor("gsc", [1, HID], F32, kind="ExternalInput")
    usc = nc.dram_tensor("usc", [1, HID], F32, kind="ExternalInput")
    dsc = nc.dram_tensor("dsc", [1, OSH], F32, kind="ExternalInput")
    at = nc.dram_tensor("at", [HID, R], BF16, kind="ExternalInput")
    bt = nc.dram_tensor("bt", [R, HID], BF16, kind="ExternalInput")
    out = nc.dram_tensor("out", [TSH, HID], BF16, kind="ExternalOutput")

    # AllGather outputs must be offset-0 gather sources -> raw Internal tensors
    cbf = [nc.dram_tensor(f"cbf{b}", [KCB, GS], BF16) for b in range(6)]

    with tile.TileContext(nc) as tc:
        with ExitStack() as ctx:
            dram = ctx.enter_context(tc.tile_pool(name="dram", bufs=1, space="DRAM"))
            xstage = dram.tile([TSH, HID], BF16)
            cbstage = [dram.tile([CBS, GS], BF16, name=f"cbst{b}")
                       for b in range(6)]
            xfull = dram.tile([TOK, HID], BF16)
            wgu = dram.tile([2 * OPAD, HID], BF16)
            wd = dram.tile([HID, OPAD], BF16)
            gu = dram.tile([TOK, 2 * OPAD], BF16)
            mid = dram.tile([TOK, OPAD], BF16)
            acc = dram.tile([TOK, HID], F32)
            lacc = dram.tile([TOK, HID], F32)
            lmidT = dram.tile([R, TOK], BF16)
            rs = dram.tile([TSH, HID], F32)

            # ---- AllGather x token shards and codebook shards ----
            nc.sync.dma_start(xstage[:], xs.ap())
            nc.gpsimd.collective_compute(
                "AllGather", mybir.AluOpType.bypass,
                replica_groups=[list(range(NC))],
                ins=[xstage[:].opt()], outs=[xfull[:].opt()])
            for b in range(6):
                nc.sync.dma_start(cbstage[b][:], cbsh[b].ap())
                nc.gpsimd.collective_compute(
                    "AllGather", mybir.AluOpType.bypass,
                    replica_groups=[list(range(NC))],
                    ins=[cbstage[b][:].opt()], outs=[cbf[b].ap().opt()])

            # ---- zero the padded weight regions ----
            with tc.tile_pool(name="zpool", bufs=1) as zpool:
                zt = zpool.tile([P, HID], BF16, tag="z")
                nc.vector.memset(zt[:], 0.0)
                npad = OPAD - OSH  # 160
                for base in (0, OPAD):
                    r = base + OSH
                    while r < base + OPAD:
                        n = min(P, base + OPAD - r)
                        nc.sync.dma_start(wgu[r:r + n, :], zt[0:n, :])
                        r += n
                for r in range(0, HID, P):
                    nc.sync.dma_start(wd[r:r + P, OSH:OPAD], zt[0:P, 0:npad])

            # ---- on-device dequantization ----
            with tc.tile_pool(name="scp", bufs=1) as scp:
                gsc_sb = scp.tile([P, HID], F32, tag="gs")
                nc.sync.dma_start(gsc_sb[:], gsc.ap().to_broadcast([P, HID]))
                usc_sb = scp.tile([P, HID], F32, tag="us")
                nc.sync.dma_start(usc_sb[:], usc.ap().to_broadcast([P, HID]))
                dsc_sb = scp.tile([P, OSH], F32, tag="ds")
                nc.sync.dma_start(dsc_sb[:], dsc.ap().to_broadcast([P, OSH]))

                _dequant(nc, tc, ctx, gi.ap(), cbf[0].ap(), cbf[1].ap(),
                         gsc_sb, wgu[:], 0, OSH, G, GS, "g")
                _dequant(nc, tc, ctx, ui.ap(), cbf[2].ap(), cbf[3].ap(),
                         usc_sb, wgu[OPAD:, :], 0, OSH, G, GS, "u")
                _dequant(nc, tc, ctx, di.ap(), cbf[4].ap(), cbf[5].ap(),
                         dsc_sb, wd[:], 0, HID, GDR, GS, "d")

            # ---- lora: lmidT = A^T(stat) x^T(mov);  lacc = lmidT^T @ B^T ----
            matmul_tile_kernel(tc,
                               kxm_ap=at.ap(),
                               kxn_ap=xfull[:],
                               mxn_ap=lmidT[:],
                               transpose_kxn=True)
            matmul_tile_kernel(tc,
                               kxm_ap=lmidT[:],
                               kxn_ap=bt.ap(),
                               mxn_ap=lacc[:])

            # ---- gate/up matmul: gu[t, 2*OPAD] = x @ Wgu^T ----
            matmul_tile_kernel(tc,
                               kxm_ap=xfull[:],
                               kxn_ap=wgu[:],
                               mxn_ap=gu[:],
                               transpose_kxm=True,
                               transpose_kxn=True)

            # ---- mid = silu(gate) * up  (bf16) ----
            with tc.tile_pool(name="si_in", bufs=3) as si_in, \
                 tc.tile_pool(name="si_t", bufs=3) as si_t, \
                 tc.tile_pool(name="si_o", bufs=3) as si_o:
                for s in range(TOK // P):
                    t0 = s * P
                    gt = si_in.tile([P, 2 * OPAD], BF16, tag="gt")
                    nc.sync.dma_start(gt[:], gu[t0:t0 + P, :])
                    sl = si_t.tile([P, OPAD], BF16, tag="sl")
                    nc.scalar.activation(sl[:], gt[:, 0:OPAD],
                                         mybir.ActivationFunctionType.Silu)
                    md = si_o.tile([P, OPAD], BF16, tag="md")
                    nc.vector.tensor_tensor(out=md[:], in0=sl[:],
                                            in1=gt[:, OPAD:2 * OPAD],
                                            op=mybir.AluOpType.mult)
                    nc.sync.dma_start(mid[t0:t0 + P, :], md[:])

            # ---- down matmul accumulated onto lora partial ----
            matmul_tile_kernel(tc,
                               kxm_ap=mid[:],
                               kxn_ap=wd[:],
                               mxn_ap=acc[:],
                               transpose_kxm=True,
                               transpose_kxn=True,
                               accumulate_ap=lacc[:])

            # ---- ReduceScatter over the 8 cores ----
            ch = TOK // RS_CHUNKS
            och = ch // NC
            for k in range(RS_CHUNKS):
                nc.gpsimd.collective_compute(
                    "ReduceScatter", mybir.AluOpType.add,
                    replica_groups=[list(range(NC))],
                    ins=[acc[k * ch:(k + 1) * ch, :].opt()],
                    outs=[rs[k * och:(k + 1) * och, :].opt()])

            # ---- emit the token shard as bf16 ----
            with tc.tile_pool(name="cv_in", bufs=3) as cv_in, \
                 tc.tile_pool(name="cv_o", bufs=3) as cv_o:
                for s in range(TSH // P):
                    t0 = s * P
                    rt = cv_in.tile([P, HID], F32, tag="rt")
                    nc.sync.dma_start(rt[:], rs[t0:t0 + P, :])
                    ot = cv_o.tile([P, HID], BF16, tag="ot")
                    nc.scalar.activation(ot[:], rt[:],
                                         mybir.ActivationFunctionType.Copy)
                    nc.sync.dma_start(out.ap()[t0:t0 + P, :], ot[:])

    nc.compile()
    return nc


def shard_inputs(cfg, inputs):
    """Build per-core in_maps from the full-size input dict."""
    d = derived(cfg)
    HID, GS, KCB, TOK, R, NC = (cfg[k] for k in
                                ("HID", "GS", "KCB", "TOK", "R", "NC"))
    G, OSH, GDR, TSH, CBS = (d[k] for k in ("G", "OSH", "GDR", "TSH", "CBS"))
    bf16 = ml_dtypes.bfloat16

    x = np.asarray(inputs["x"], np.float32).reshape(TOK, HID)
    xb = np.ascontiguousarray(x.astype(bf16))

    cbs = [np.asarray(inputs["gate_codebooks"], np.float32)[0],
           np.asarray(inputs["gate_codebooks"], np.float32)[1],
           np.asarray(inputs["up_codebooks"], np.float32)[0],
           np.asarray(inputs["up_codebooks"], np.float32)[1],
           np.asarray(inputs["down_codebooks"], np.float32)[0],
           np.asarray(inputs["down_codebooks"], np.float32)[1]]
    cbs = [np.ascontiguousarray(c.astype(bf16)) for c in cbs]
    gi = np.asarray(inputs["gate_indices"], np.int32)
    ui = np.asarray(inputs["up_indices"], np.int32)
    di = np.asarray(inputs["down_indices"], np.int32)
    gs_ = np.asarray(inputs["gate_scales"], np.float32).reshape(1, HID)
    us_ = np.asarray(inputs["up_scales"], np.float32).reshape(1, HID)
    ds_ = np.asarray(inputs["down_scales"], np.float32)
    at = np.ascontiguousarray(np.asarray(inputs["lora_A"], np.float32).T.astype(bf16))
    SCALING = 256.0 / 128.0
    bt = np.ascontiguousarray(
        (np.asarray(inputs["lora_B"], np.float32).T * (SCALING / NC)).astype(bf16))

    def pack_idx(idx):
        # [rows, Gn, 2] int32 -> [rows, 2*Gn] uint16 (book0 cols then book1)
        return np.ascontiguousarray(
            np.concatenate([idx[:, :, 0], idx[:, :, 1]], axis=1).astype(np.uint16))

    in_maps = []
    for c in range(NC):
        m = {
            "xs": np.ascontiguousarray(xb[c * TSH:(c + 1) * TSH]),
            "gi": pack_idx(gi[c * OSH:(c + 1) * OSH]),
            "ui": pack_idx(ui[c * OSH:(c + 1) * OSH]),
            "di": pack_idx(di[:, c * GDR:(c + 1) * GDR, :]),
            "gsc": gs_, "usc": us_,
            "dsc": np.ascontiguousarray(
                (ds_[c * OSH:(c + 1) * OSH] * 0.01).reshape(1, OSH)),
            "at": at, "bt": bt,
        }
        for b in range(6):
            m[f"cbsh{b}"] = np.ascontiguousarray(cbs[b][c * CBS:(c + 1) * CBS])
        in_maps.append(m)
    return in_maps


_NC_CACHE = {}
_RUNNER_CACHE = {}


def _compiled(cfg):
    key = tuple(sorted(cfg.items()))
    if key not in _NC_CACHE:
        _NC_CACHE[key] = build(cfg)
    return _NC_CACHE[key]


class Runner:
    """Dispatch a compiled Bass SPMD kernel across cores via PJRT, shipping
    each core's shard directly (no host-side concat) and, in nki mode, no
    zero-filled output donation buffers."""

    def __init__(self, nc, n_cores):
        import jax
        from jax.sharding import Mesh, PartitionSpec, NamedSharding
        from jax.experimental.shard_map import shard_map
        from concourse.bass2jax import (_bass_exec_p, install_neuronx_cc_hook,
                                        partition_id_tensor)

        install_neuronx_cc_hook()
        self.jax = jax
        self.nc = nc
        self.n_cores = n_cores
        partition_name = (nc.partition_id_tensor.name
                          if nc.partition_id_tensor else None)
        in_names, out_names, out_avals, zero_outs = [], [], [], []
        for alloc in nc.m.functions[0].allocations:
            if not isinstance(alloc, mybir.MemoryLocationSet):
                continue
            name = alloc.memorylocations[0].name
            if alloc.kind == "ExternalInput":
                if name != partition_name:
                    in_names.append(name)
            elif alloc.kind == "ExternalOutput":
                out_names.append(name)
                shape = tuple(alloc.tensor_shape)
                dtype = mybir.dt.np(alloc.dtype)
                out_avals.append(jax.core.ShapedArray(shape, dtype))
                zero_outs.append(np.zeros(shape, dtype))
        self.in_names = in_names
        self.out_names = out_names
        self.out_avals = out_avals
        self.zero_outs = zero_outs if MODE == "exec" else []
        n_params = len(in_names)
        n_outs = len(out_names)

        bind_in_names = list(in_names)
        if MODE == "exec":
            bind_in_names.extend(out_names)
        if partition_name is not None:
            bind_in_names.append(partition_name)

        def _body(*args):
            operands = list(args)
            if partition_name is not None:
                operands.append(partition_id_tensor())
            outs = _bass_exec_p.bind(
                *operands,
                out_avals=tuple(out_avals),
                in_names=tuple(bind_in_names),
                out_names=tuple(out_names),
                lowering_input_output_aliases=(),
                sim_require_finite=True,
                sim_require_nnan=True,
                nc=nc,
            )
            return tuple(outs)

        self.devices = jax.devices()[:n_cores]
        self.mesh = Mesh(np.asarray(self.devices), ("core",))
        self.pspec = PartitionSpec("core")
        self.sharding = NamedSharding(self.mesh, self.pspec)
        n_all = n_params + (n_outs if MODE == "exec" else 0)
        in_specs = (self.pspec,) * n_all
        out_specs = (self.pspec,) * n_outs
        donate = (tuple(range(n_params, n_params + n_outs))
                  if MODE == "exec" else ())
        self.fn = jax.jit(
            shard_map(_body, mesh=self.mesh, in_specs=in_specs,
                      out_specs=out_specs, check_rep=False),
            donate_argnums=donate, keep_unused=True)

    def _to_global(self, shards):
        jax = self.jax
        s = shards[0].shape
        global_shape = (self.n_cores * s[0],) + tuple(s[1:])
        parts = [jax.device_put(shards[c], self.devices[c])
                 for c in range(self.n_cores)]
        return jax.make_array_from_single_device_arrays(
            global_shape, self.sharding, parts)

    def __call__(self, in_maps):
        jax = self.jax
        args = [self._to_global([m[name] for m in in_maps])
                for name in self.in_names]
        for z in self.zero_outs:
            args.append(self._to_global(
                [z] + [np.zeros_like(z) for _ in range(self.n_cores - 1)]))
        out_arrs = self.fn(*args)
        out_arrs = jax.block_until_ready(out_arrs)
        res = [{} for _ in range(self.n_cores)]
        for i, name in enumerate(self.out_names):
            g = np.asarray(out_arrs[i])
            s0 = self.out_avals[i].shape[0]
            for c in range(self.n_cores):
                res[c][name] = g[c * s0:(c + 1) * s0]
        return res


def _runner(cfg):
    key = tuple(sorted(cfg.items()))
    if key not in _RUNNER_CACHE:
        _RUNNER_CACHE[key] = Runner(_compiled(cfg), cfg["NC"])
    return _RUNNER_CACHE[key]


def assemble(cfg, results):
    """Reorder the chunked-ReduceScatter per-core shards into token order."""
    TOK, NC, HID = cfg["TOK"], cfg["NC"], cfg["HID"]
    ch = TOK // RS_CHUNKS
    och = ch // NC
    outs = np.empty((TOK, HID), np.float32)
    for c in range(NC):
        p = results[c]["out"].astype(np.float32)
        for k in range(RS_CHUNKS):
            outs[k * ch + c * och:k * ch + (c + 1) * och] = p[k * och:(k + 1) * och]
    return outs


def run(cfg, inputs):
    runner = _runner(cfg)
    in_maps = shard_inputs(cfg, inputs)
    res = runner(in_maps)
    return assemble(cfg, res), res


def kernel(**inputs):
    cfg = full_cfg()
    x = np.asarray(inputs["x"])
    outs, _ = run(cfg, inputs)
    return outs.reshape(x.shape[0], x.shape[1], cfg["HID"]).astype(np.float32)
